# revision 28
# baseline (speedup 1.0000x reference)
"""Fused transformer block (attention + MLP) on 8 trn2 NeuronCores.

Sharding (8-way, batch-symmetric): every core computes attention for ONE
head-pair (heads 2c, 2c+1) of BOTH batches, and owns a 256-token shard of
each batch (tokens [256c, 256c+256)). The two shard halves are concatenated
along the free axis into one 512-column tile set, so projection + FFN code
is identical to a single 512-token shard.

Schedule: the attention inner loop is software-pipelined at k-chunk-pair
granularity: scores(b2i) -> exp(b2i) on the scalar engine -> weiv(b2i)
trailing one chunk behind, with qk / v-tile units for the NEXT q-tile (or
next batch) interleaved as tensor-engine filler so the PE never idles long
enough for the HAM clock gate to re-throttle. PSUM: scores ring 2x2 banks,
weiv accumulators 2x1, qk/v/rb scratch ring 2x1 = 8 banks.

All weight/bias loads are batched host-side (wp 1 DMA, biases 3, xs 1) and
issued from the sync queue (hardware DGE) instead of gpsimd (software DGE,
~1us per descriptor). gpsimd keeps only memsets, collectives and the a2a
receive DMAs. a2a staging DMAs ride the vector queue right behind the
normalization muls that produce them.

FFN: FFN2 is split 6+2 (not 4+4): 6 column blocks accumulate interleaved
inside the FFN1 loop (PSUM: 2 FFN1 ring + 6 accumulators), the last 2 run
in a short tail that reuses the FFN1 ring slots (no pool-close bubble) and
whose w2 slices are fully preloaded during batch-1 attention.
"""

import sys

for _p in ("/opt/trn_rl_repo",):
    if _p not in sys.path:
        sys.path.append(_p)

import numpy as np
import ml_dtypes

import concourse.bass as bass
import concourse.tile as tile
from concourse import bacc, mybir
from concourse.bass_utils import run_bass_kernel_spmd

BF16 = mybir.dt.bfloat16
F32 = mybir.dt.float32
AF = mybir.ActivationFunctionType
ALU = mybir.AluOpType

N_CORES = 8
B, T, C = 2, 2048, 1024
H, HS = 16, 64
F = 4 * C
TS = 512          # per-core fused shard width (256 tokens x 2 batches)
TSB = 256         # per-batch shard width
CC = C // 128
FB = F // 128
SCALE = float(C) ** -0.5
NA = 6            # FFN2 group-A column blocks (interleaved in FFN1)


def build_program(nc: bass.Bass):
    xt_bf = nc.dram_tensor("xt_bf", [B, CC, 128, T], BF16,
                           kind="ExternalInput").ap()
    xs_d = nc.dram_tensor("xs", [128, CC * TS], F32, kind="ExternalInput").ap()
    wq_d = nc.dram_tensor("wq", [128, C], BF16, kind="ExternalInput").ap()
    wk_d = nc.dram_tensor("wk", [128, C], BF16, kind="ExternalInput").ap()
    wv_d = nc.dram_tensor("wv", [128, C], BF16, kind="ExternalInput").ap()
    wp_d = nc.dram_tensor("wp", [128, 8 * CC * 128], BF16,
                          kind="ExternalInput").ap()
    w1_d = nc.dram_tensor("w1", [CC, 128, F], BF16, kind="ExternalInput").ap()
    w2_d = nc.dram_tensor("w2", [FB, 128, C], BF16, kind="ExternalInput").ap()
    bp_d = nc.dram_tensor("bp", [128, CC], F32, kind="ExternalInput").ap()
    b1_d = nc.dram_tensor("b1", [128, FB], F32, kind="ExternalInput").ap()
    b2_d = nc.dram_tensor("b2", [128, CC], F32, kind="ExternalInput").ap()
    out_d = nc.dram_tensor("outT", [CC, 128, TS], F32, kind="ExternalOutput").ap()

    with tile.TileContext(nc) as tc:
        _emit(nc, tc, xt_bf, xs_d, wq_d, wk_d, wv_d, wp_d, w1_d, w2_d,
              bp_d, b1_d, b2_d, out_d)


def _emit(nc, tc, xt_bf, xs_d, wq_d, wk_d, wv_d, wp_d, w1_d, w2_d,
          bp_d, b1_d, b2_d, out_d):
    from contextlib import ExitStack

    ctx = ExitStack()
    with ctx:
        st = ctx.enter_context(tc.tile_pool(name="static", bufs=1))
        big = ctx.enter_context(tc.tile_pool(name="big", bufs=16))
        expp = ctx.enter_context(tc.tile_pool(name="expp", bufs=4))
        w2p = ctx.enter_context(tc.tile_pool(name="w2s", bufs=3))
        w2bp = ctx.enter_context(tc.tile_pool(name="w2bp", bufs=10))
        outp = ctx.enter_context(tc.tile_pool(name="outp", bufs=2))
        rcp = ctx.enter_context(tc.tile_pool(name="rcp", bufs=1))
        stgp = ctx.enter_context(tc.tile_pool(name="stgp", bufs=2))

        # attention PSUM pools (closed before proj)
        ps_ctx = tc.tile_pool(name="ps", bufs=2, space="PSUM")
        ps = ps_ctx.__enter__()
        sc_ctx = tc.tile_pool(name="scps", bufs=2, space="PSUM")
        scp = sc_ctx.__enter__()
        wv_ctx = tc.tile_pool(name="wvps", bufs=2, space="PSUM")
        wvp = wv_ctx.__enter__()

        a2a_in = [nc.dram_tensor(f"a2a_in{b}", [8 * 128, TSB], BF16,
                                 kind="Internal").ap() for b in range(B)]
        a2a_out = [nc.dram_tensor(f"a2a_out{b}", [8 * 128, TSB], BF16,
                                  kind="Internal").ap() for b in range(B)]
        RG8 = [[0, 1, 2, 3, 4, 5, 6, 7]]

        # ---- gpsimd-cheap setup first: memsets (no DMA deps) ----
        ones1 = st.tile([1, 64], BF16, tag="ones1", name="ones1")
        nc.gpsimd.memset(ones1[:], 1.0)
        mask_big = st.tile([128, 896], BF16, tag="mask", name="mask_big")
        nc.gpsimd.memset(mask_big[:], 1.0)
        nc.gpsimd.affine_select(mask_big[:], mask_big[:], pattern=[[1, 896]],
                                compare_op=ALU.is_ge, fill=0.0, base=-384,
                                channel_multiplier=-1)
        v_sb = [[None] * (T // 128) for _ in range(B)]
        for b in range(B):
            for tk in range(T // 128):
                vt = st.tile([128, 2 * 65], BF16, tag=f"v{b}_{tk}",
                             name=f"v_sb{b}_{tk}")
                nc.gpsimd.memset(vt[:], 1.0)
                v_sb[b][tk] = vt

        # ---- warm-up matmuls: run during the initial input DMA window
        # ---- (mask_big doubles as the warm-up operand) ----
        for wi in range(2):
            acc = ps.tile([128, 512], F32, tag="ps", name=f"wu{wi}")
            for _ in range(18):
                nc.tensor.matmul(acc[:], mask_big[:, 0:128],
                                 mask_big[:, 128:640], start=True, stop=True)

        # ---- input loads: all on the sync queue (hardware DGE) ----
        xt_sb = [[None] * CC for _ in range(B)]
        for b in range(B):
            for cc in range(CC):
                xt_sb[b][cc] = big.tile([128, T], BF16, tag="big",
                                        name=f"xt_sb{b}_{cc}")
        # qkv weights first (small, needed by the very first matmuls),
        # then batch-0 x t-chunk-major so qt=0 work can start early
        wqkv_t = {}
        for nm, d_ in (("k", wk_d), ("q", wq_d), ("v", wv_d)):
            t_ = st.tile([128, C], BF16, tag=f"w{nm}", name=f"w{nm}_t")
            nc.sync.dma_start(t_[:], d_)
            wqkv_t[nm] = t_
        for q4 in range(4):
            for cc in range(CC):
                nc.sync.dma_start(xt_sb[0][cc][:, q4 * 512:(q4 + 1) * 512],
                                  xt_bf[0, cc][:, q4 * 512:(q4 + 1) * 512])
        for cc in range(CC):
            nc.sync.dma_start(xt_sb[1][cc][:], xt_bf[1, cc][:])
        wp_t = st.tile([128, 8 * CC * 128], BF16, tag="wp", name="wp_t")
        nc.sync.dma_start(wp_t[:], wp_d)
        bp_t = st.tile([128, CC], F32, tag="bp", name="bp_t")
        nc.sync.dma_start(bp_t[:], bp_d)
        b1_t = st.tile([128, FB], F32, tag="b1", name="b1_t")
        nc.sync.dma_start(b1_t[:], b1_d)
        b2_t = st.tile([128, CC], F32, tag="b2", name="b2_t")
        nc.sync.dma_start(b2_t[:], b2_d)

        qT = [st.tile([128, T], BF16, tag=f"qT{b}", name=f"qT_sb{b}")
              for b in range(B)]
        kT = [st.tile([128, T], BF16, tag=f"kT{b}", name=f"kT_sb{b}")
              for b in range(B)]
        exq = {}  # (b, qt, hh, b2i) -> exp tile

        # ---------- unit emitters ----------
        def qk_unit(b, nm, tt):
            # one 512-token slice of qT/kT for batch b
            dst = kT[b] if nm == "k" else qT[b]
            acc = ps.tile([128, 512], F32, tag="ps", name=f"pqk_{nm}{b}{tt}")
            w_t = wqkv_t[nm]
            for cc in range(CC):
                nc.tensor.matmul(
                    acc[:],
                    w_t[:, cc * 128:(cc + 1) * 128],
                    xt_sb[b][cc][:, tt * 512:(tt + 1) * 512],
                    start=(cc == 0), stop=(cc == CC - 1),
                )
            nc.vector.tensor_copy(dst[:, tt * 512:(tt + 1) * 512], acc[:])

        def v_unit(b, tk):
            vt = v_sb[b][tk]
            acc = ps.tile([128, 128], F32, tag="ps", name=f"ps_v{b}_{tk}")
            for cc in range(CC):
                nc.tensor.matmul(
                    acc[:],
                    xt_sb[b][cc][:, tk * 128:(tk + 1) * 128],
                    wqkv_t["v"][:, cc * 128:(cc + 1) * 128],
                    start=(cc == 0), stop=(cc == CC - 1),
                )
            src = acc.rearrange("p (h d) -> p h d", h=2)
            dstv = vt.rearrange("p (h d) -> p h d", h=2, d=65)[:, :, 0:64]
            nc.vector.tensor_copy(dstv, src)

        def s_unit(b, qt, b2i):
            # scores + exp (+ causal mask) for k-chunks 2*b2i, 2*b2i+1.
            # The two hh matmuls sit in disjoint PE row groups (partitions
            # 0-63 vs 64-127) so they run concurrently.
            sc = [None, None]
            for hh in range(2):
                sc[hh] = scp.tile([128, 1024], F32, tag="sc",
                                  name=f"psc{b}{hh}{qt}_{b2i}")
            for j in range(2):
                kc = 2 * b2i + j
                for hh in range(2):
                    p0 = 64 * hh
                    nc.tensor.matmul(
                        sc[hh][:, j * 512:(j + 1) * 512],
                        kT[b][p0:p0 + 64, kc * 128:(kc + 1) * 128],
                        qT[b][p0:p0 + 64, qt * 512:(qt + 1) * 512],
                        start=True, stop=True,
                    )
            for hh in range(2):
                ext = expp.tile([128, 1024], BF16, tag="expp",
                                name=f"ex{b}{hh}{qt}_{b2i}")
                nc.scalar.activation(ext[:], sc[hh][:], AF.Exp, scale=SCALE)
                for j in range(2):
                    kc = 2 * b2i + j
                    if kc >= 4 * qt:
                        dd = kc * 128 - qt * 512
                        nc.vector.tensor_mul(
                            ext[:, j * 512:(j + 1) * 512],
                            ext[:, j * 512:(j + 1) * 512],
                            mask_big[:, 384 - dd:896 - dd],
                        )
                exq[(b, qt, hh, b2i)] = ext

        def w_unit(b, qt, b2i, wv_acc):
            nkc = 4 * (qt + 1)
            for j in range(2):
                kc = 2 * b2i + j
                for hh in range(2):
                    ext = exq[(b, qt, hh, b2i)]
                    nc.tensor.matmul(
                        wv_acc[hh][:],
                        v_sb[b][kc][:, hh * 65:hh * 65 + 65],
                        ext[:, j * 512:(j + 1) * 512],
                        start=(kc == 0), stop=(kc == nkc - 1),
                    )

        def f_unit(b, qt, wv_acc):
            # normalize by the softmax denominator (the ones-column of V)
            stg = stgp.tile([128, 512], BF16, tag="stg", name=f"stg{b}{qt}")
            for hh in range(2):
                p0 = 64 * hh
                den = rcp.tile([1, 512], F32, tag="den", name=f"den{b}{hh}{qt}")
                nc.vector.tensor_copy(den[:], wv_acc[hh][64:65, :])
                rc = rcp.tile([1, 512], F32, tag="rc", name=f"rc{b}{hh}{qt}")
                nc.vector.reciprocal_approx_fast(rc[:], den[:])
                rcb = rcp.tile([1, 512], BF16, tag="rcb", name=f"rcb{b}{hh}{qt}")
                nc.vector.tensor_copy(rcb[:], rc[:])
                rb = ps.tile([64, 512], F32, tag="ps", name=f"rb{b}{hh}{qt}")
                nc.tensor.matmul(rb[:], ones1[:], rcb[:], start=True, stop=True)
                rbs = rcp.tile([64, 512], BF16, tag="rbs", name=f"rbs{b}{hh}{qt}")
                nc.vector.tensor_copy(rbs[:], rb[:])
                nc.vector.tensor_mul(stg[p0:p0 + 64, :], wv_acc[hh][0:64, :],
                                     rbs[:])
            # stage the two dest-shard chunks for the AllToAll (gpsimd queue
            # so they don't sit behind weight loads on the sync DMA queue)
            for j in range(2):
                s = 2 * qt + j
                nc.gpsimd.dma_start(
                    a2a_in[b][s * 128:(s + 1) * 128, :],
                    stg[:, j * TSB:(j + 1) * TSB])

        # ---------- attention schedule ----------
        # fillers(b, qt) = prerequisite units of the NEXT qt block, emitted
        # as PE filler between the scalar-gated s/w steps of this block.
        fillers = {
            (0, 0): [("qk", 0, "k", 1), ("qk", 0, "q", 1)] +
                    [("v", 0, tk) for tk in range(4, 8)],
            (0, 1): [("qk", 0, "k", 2), ("qk", 0, "q", 2)] +
                    [("v", 0, tk) for tk in range(8, 12)],
            (0, 2): [("qk", 0, "k", 3), ("qk", 0, "q", 3)] +
                    [("v", 0, tk) for tk in range(12, 16)],
            (0, 3): [("qk", 1, "k", 0), ("qk", 1, "q", 0)] +
                    [("v", 1, tk) for tk in range(0, 4)],
            (1, 0): [("qk", 1, "k", 1), ("qk", 1, "q", 1)] +
                    [("v", 1, tk) for tk in range(4, 8)],
            (1, 1): [("qk", 1, "k", 2), ("qk", 1, "q", 2)] +
                    [("v", 1, tk) for tk in range(8, 12)],
            (1, 2): [("qk", 1, "q", 3), ("v", 1, 12), ("v", 1, 13)],
            (1, 3): [("qk", 1, "k", 3), ("v", 1, 14), ("v", 1, 15)],
        }

        def emit_filler(u):
            if u[0] == "qk":
                qk_unit(u[1], u[2], u[3])
            else:
                v_unit(u[1], u[2])

        def qt_block(b, qt, carry, post=None):
            # software pipeline with a TWO-chunk weiv lag: W(qt,i) is emitted
            # after S(qt,i+2), so the PE always has ~2 exp-latencies of
            # independent work queued ahead of each exp-gated weiv unit. The
            # previous block's last two weiv units (+ its finalize, whose rb
            # broadcast matmuls wait on a short DVE chain) are carried into
            # the head of this block for the same reason.
            nb2i = 2 * (qt + 1)
            fl = list(fillers[(b, qt)])
            # for (1,3) the fillers are this block's OWN late prerequisites:
            # kT(1,3) is only needed from b2i=6, v(1,14/15) from b2i=7.
            own_late = (b, qt) == (1, 3)
            wv_acc = [wvp.tile([65, 512], F32, tag="wv",
                               name=f"pwv{b}{hh}{qt}")
                      for hh in range(2)]
            for i in range(nb2i):
                if own_late:
                    if i == 2 and fl:
                        emit_filler(fl.pop(0))       # kT(1,3)
                    if i == 4 and len(fl) == 2:
                        emit_filler(fl.pop(0))       # v(1,14)
                        emit_filler(fl.pop(0))       # v(1,15)
                s_unit(b, qt, i)
                if carry:
                    carry.pop(0)()
                elif i >= 2:
                    w_unit(b, qt, i - 2, wv_acc)
                if i <= 1 and len(fl) < 3:
                    wacc = ps.tile([128, 512], F32, tag="ps",
                                   name=f"wf{b}{qt}{i}")
                    nc.tensor.matmul(wacc[:], mask_big[:, 0:128],
                                     mask_big[:, 128:640],
                                     start=True, stop=True)
                if not own_late:
                    if fl:
                        emit_filler(fl.pop(0))
                    if fl and (i <= 1 or len(fl) > nb2i - 1 - i):
                        emit_filler(fl.pop(0))
            while fl:
                emit_filler(fl.pop(0))
            while carry:
                carry.pop(0)()
            if post is not None:
                post()
            return [
                lambda: w_unit(b, qt, nb2i - 2, wv_acc),
                lambda: (w_unit(b, qt, nb2i - 1, wv_acc),
                         f_unit(b, qt, wv_acc)),
            ]

        # batch 0 prerequisites
        qk_unit(0, "k", 0)
        qk_unit(0, "q", 0)
        for tk in range(4):
            v_unit(0, tk)

        carry = []
        for qt in range(4):
            carry = qt_block(0, qt, carry)
        # qt_block(1,0) flushes f(0,3) at its head, so the batch-0 staging
        # DMAs precede the first collective on the gpsimd queue
        carry = qt_block(1, 0, carry)

        nc.gpsimd.collective_compute(
            "AllToAll", ALU.bypass, replica_groups=RG8,
            ins=[a2a_in[0]], outs=[a2a_out[0]],
        )
        rt = [st.tile([128, 8 * TSB], BF16, tag=f"rt{b}", name=f"rt{b}")
              for b in range(B)]

        def emit_rcv(b):
            # one gather-descriptor receive on the sync queue: rt[b][p,
            # s*256+j] <- a2a_out[b][s*128+p, j]. No 8x staggered issue
            # latency, and it never delays the gpsimd staging/collective
            # chain (transfers queued behind it on sync are needed later
            # than the exchange completes).
            nc.sync.dma_start(
                rt[b].rearrange("p (s j) -> p s j", s=8),
                a2a_out[b].rearrange("(s p) j -> p s j", s=8))

        def emit_w1_loads():
            # stream during batch-1 attention on the sync queue
            for half in range(2):
                for cc in range(CC):
                    t_ = big.tile([128, 2048], BF16, tag="big",
                                  name=f"w1_sb{cc}_{half}")
                    nc.sync.dma_start(
                        t_[:], w1_d[cc][:, half * 2048:(half + 1) * 2048])
                    w1_sb[cc][half] = t_

        w1_sb = [[None, None] for _ in range(CC)]

        def post_11():
            emit_rcv(0)
            emit_w1_loads()
            # xs (fp32 residual): right behind w1 on the sync queue, done
            # well before the proj residual-adds need it
            for xi in range(4):
                xst = big.tile([128, 1024], F32, tag="xsb", bufs=4,
                               name=f"xs{xi}")
                nc.sync.dma_start(xst[:], xs_d[:, xi * 1024:(xi + 1) * 1024])
                xs_sb.append(xst)

        xs_sb = []
        carry = qt_block(1, 1, carry, post=post_11)
        carry = qt_block(1, 2, carry)
        carry = qt_block(1, 3, carry)
        while carry:
            carry.pop(0)()  # flush W(1,3,6/7) + f(1,3): the a2a needs them

        wv_ctx.__exit__(None, None, None)
        sc_ctx.__exit__(None, None, None)
        ps_ctx.__exit__(None, None, None)

        # ---- proj (streamed): per cb, 8 matmuls into a ring-2 PSUM tile,
        # residual-add immediately, release. Batch-0 columns only need the
        # first exchange; together with the batch-0 halves of the first NFH
        # FFN1 row-blocks they keep the PE busy for the whole span of the
        # batch-1 AllToAll.
        pf1_ctx = tc.tile_pool(name="pf1", bufs=2, space="PSUM")
        pf1 = pf1_ctx.__enter__()
        pjp_ctx = tc.tile_pool(name="pjp", bufs=2, space="PSUM")
        pjp = pjp_ctx.__enter__()

        def xs_slice(cb):
            return xs_sb[cb // 2][:, (cb % 2) * 512:(cb % 2) * 512 + 512]

        x1b = [st.tile([128, TS], BF16, tag=f"x1b{cb}", name=f"x1b{cb}")
               for cb in range(CC)]

        def proj_cb(cb, h0):
            acc = pjp.tile([128, TSB], F32, tag="pj", name=f"ps_pj{cb}_{h0}")
            for s in range(8):
                nc.tensor.matmul(
                    acc[:],
                    wp_t[:, (s * CC + cb) * 128:(s * CC + cb + 1) * 128],
                    rt[h0][:, s * TSB:(s + 1) * TSB],
                    start=(s == 0), stop=(s == 7),
                )
            nc.vector.scalar_tensor_tensor(
                x1b[cb][:, h0 * TSB:(h0 + 1) * TSB], acc[:],
                bp_t[:, cb:cb + 1],
                xs_slice(cb)[:, h0 * TSB:(h0 + 1) * TSB], ALU.add, ALU.add)

        hT = [st.tile([128, TS], BF16, tag=f"hT{fb}", name=f"hT{fb}")
              for fb in range(FB)]
        # every FFN1 row-block is computed in batch-halves: the batch-0
        # halves (which only need the first exchange) hide the entire
        # batch-1 AllToAll; a half-pair costs the same as one full-width
        # block (the N=256 matmuls still cover the LDWEIGHTS)

        def ffn1_half(fb, h0):
            w1h, fo = fb // 16, fb % 16
            acc = pf1.tile([128, TSB], F32, tag="pf1", name=f"ps_h{fb}_{h0}")
            for cc in range(CC):
                nc.tensor.matmul(
                    acc[:],
                    w1_sb[cc][w1h][:, fo * 128:(fo + 1) * 128],
                    x1b[cc][:, h0 * TSB:(h0 + 1) * TSB],
                    start=(cc == 0), stop=(cc == CC - 1))
            nc.scalar.activation(hT[fb][:, h0 * TSB:(h0 + 1) * TSB], acc[:],
                                 AF.Relu, bias=b1_t[:, fb:fb + 1])

        for cb in range(CC):
            proj_cb(cb, 0)

        nc.gpsimd.collective_compute(
            "AllToAll", ALU.bypass, replica_groups=RG8,
            ins=[a2a_in[1]], outs=[a2a_out[1]],
        )
        emit_rcv(1)

        for fb in range(FB):
            ffn1_half(fb, 0)

        for cb in range(CC):
            proj_cb(cb, 1)
        pjp_ctx.__exit__(None, None, None)

        # ---- FFN1 (remaining) with FFN2 group A (cb 0..NA-1) interleaved
        pfa_ctx = tc.tile_pool(name="pfa", bufs=NA, space="PSUM")
        pfa = pfa_ctx.__enter__()
        accA = [pfa.tile([128, TS], F32, tag=f"pfa{cb}", bufs=1,
                         name=f"ps_oa{cb}") for cb in range(NA)]
        w2b_sb = []

        def ffn2a_mms(fb):
            wt = w2p.tile([128, NA * 128], BF16, tag="w2s", name=f"w2ta{fb}")
            nc.sync.dma_start(wt[:], w2_d[fb][:, 0:NA * 128])
            if fb >= FB - 10:
                # prefetch the first 8 group-B w2 slices during the FFN1 tail
                wtb = w2bp.tile([128, (CC - NA) * 128], BF16, tag="w2b",
                                name=f"w2tb{fb - (FB - 10)}")
                nc.sync.dma_start(wtb[:], w2_d[fb - (FB - 10)][:, NA * 128:C])
                w2b_sb.append(wtb)
            for cb in range(NA):
                nc.tensor.matmul(
                    accA[cb][:], wt[:, cb * 128:(cb + 1) * 128], hT[fb][:],
                    start=(fb == 0), stop=(fb == FB - 1))

        for fb in range(FB):
            ffn1_half(fb, 1)
            ffn2a_mms(fb)

        # ---- FFN2 group B (cb NA..7): reuses the pf1 ring slots; w2
        # slices beyond the prefetched 8 stream just-in-time ----
        accB = [pf1.tile([128, TS], F32, tag="pf1", name=f"ps_ob{cb}")
                for cb in range(CC - NA)]
        first = True
        for fc in range(FB):
            wt = w2b_sb[fc]
            for cb in range(CC - NA):
                nc.tensor.matmul(
                    accB[cb][:], wt[:, cb * 128:(cb + 1) * 128], hT[fc][:],
                    start=(fc == 0), stop=(fc == FB - 1))
            if fc + 10 < FB:
                wtb = w2bp.tile([128, (CC - NA) * 128], BF16, tag="w2b",
                                name=f"w2tb{fc + 10}")
                nc.sync.dma_start(wtb[:], w2_d[fc + 10][:, NA * 128:C])
                w2b_sb.append(wtb)
            if first:
                first = False
                # group-A outputs drain while B accumulates
                for cb in range(NA):
                    ot = outp.tile([128, TS], F32, tag="outp", name=f"ot{cb}")
                    nc.vector.scalar_tensor_tensor(
                        ot[:], accA[cb][:], b2_t[:, cb:cb + 1],
                        x1b[cb][:], ALU.add, ALU.add)
                    nc.scalar.dma_start(out_d[cb], ot[:])
        for cb4 in range(CC - NA):
            cb = cb4 + NA
            ot = outp.tile([128, TS], F32, tag="outp", name=f"ot{cb}")
            nc.vector.scalar_tensor_tensor(ot[:], accB[cb4][:],
                                           b2_t[:, cb:cb + 1],
                                           x1b[cb][:], ALU.add, ALU.add)
            nc.scalar.dma_start(out_d[cb], ot[:])

        pfa_ctx.__exit__(None, None, None)
        pf1_ctx.__exit__(None, None, None)


_CACHED = None


def _get_compiled():
    global _CACHED
    if _CACHED is None:
        nc = bacc.Bacc("TRN2", target_bir_lowering=False, debug=False,
                       num_devices=N_CORES)
        build_program(nc)
        nc.compile()
        _CACHED = nc
    return _CACHED


def _prep_inputs(x, Wq, Wk, Wv, Wproj, bproj, W1, b1, W2, b2):
    bf = ml_dtypes.bfloat16
    W1t = np.ascontiguousarray(W1.astype(bf).reshape(CC, 128, F))
    W2t = np.ascontiguousarray(W2.astype(bf).reshape(FB, 128, C))
    b1r = np.ascontiguousarray(b1.astype(np.float32).reshape(FB, 128).T)
    b2r = np.ascontiguousarray(b2.astype(np.float32).reshape(CC, 128).T)
    bpr = np.ascontiguousarray(bproj.astype(np.float32).reshape(CC, 128).T)
    # full Wproj on every core: wp_flat[p, (s*CC+cb)*128+k] = Wproj[128s+p, 128cb+k]
    wp_flat = np.ascontiguousarray(
        Wproj.astype(bf).reshape(8, 128, CC, 128).transpose(1, 0, 2, 3)
        .reshape(128, 8 * CC * 128))
    # x transposed, both batches, shared by all cores
    xT = [np.ascontiguousarray(x[b].T.astype(np.float32)) for b in range(B)]
    xT_bf = np.ascontiguousarray(
        np.stack([xT[b].astype(bf).reshape(CC, 128, T) for b in range(B)]))

    in_maps = []
    for c in range(N_CORES):
        cols = slice(128 * c, 128 * (c + 1))
        # wq_t[p, cc*128+k] = Wq[128cc+p, core_cols[k]]
        wq_s = np.ascontiguousarray(
            Wq[:, cols].astype(bf).reshape(CC, 128, 128)
            .transpose(1, 0, 2).reshape(128, C))
        wk_s = np.ascontiguousarray(
            Wk[:, cols].astype(bf).reshape(CC, 128, 128)
            .transpose(1, 0, 2).reshape(128, C))
        wv_s = np.ascontiguousarray(
            Wv[:, cols].astype(bf).reshape(CC, 128, 128)
            .transpose(1, 0, 2).reshape(128, C))
        tok = slice(TSB * c, TSB * (c + 1))
        # xs_t[p, cb*512+j] = fused-shard residual, fp32
        xts = np.ascontiguousarray(
            np.concatenate([xT[0][:, tok], xT[1][:, tok]], axis=1)
            .reshape(CC, 128, TS).transpose(1, 0, 2).reshape(128, CC * TS))
        in_maps.append({
            "xt_bf": xT_bf, "xs": xts,
            "wq": wq_s, "wk": wk_s, "wv": wv_s, "wp": wp_flat,
            "w1": W1t, "w2": W2t, "bp": bpr, "b1": b1r, "b2": b2r,
        })
    return in_maps


def kernel(x, Wq, Wk, Wv, Wproj, bproj, W1, b1, W2, b2, _trace=False):
    nc = _get_compiled()
    in_maps = _prep_inputs(np.asarray(x), np.asarray(Wq), np.asarray(Wk),
                           np.asarray(Wv), np.asarray(Wproj), np.asarray(bproj),
                           np.asarray(W1), np.asarray(b1), np.asarray(W2),
                           np.asarray(b2))
    res = run_bass_kernel_spmd(nc, in_maps, list(range(N_CORES)), trace=_trace)
    out = np.empty((B, T, C), dtype=np.float32)
    for c in range(N_CORES):
        shard = res.results[c]["outT"].reshape(C, TS)
        for b in range(B):
            out[b, TSB * c: TSB * (c + 1), :] = shard[:, TSB * b:TSB * (b + 1)].T
    if _trace:
        kernel.last_exec_time_ns = res.exec_time_ns
    return out


# revision 29
# speedup vs baseline: 1.0201x; 1.0201x over previous
"""Fused transformer block (attention + MLP) on 8 trn2 NeuronCores.

Sharding (8-way, batch-symmetric): every core computes attention for ONE
head-pair (heads 2c, 2c+1) of BOTH batches, and owns a 256-token shard of
each batch (tokens [256c, 256c+256)). The two shard halves are concatenated
along the free axis into one 512-column tile set, so projection + FFN code
is identical to a single 512-token shard.

Schedule: the attention inner loop is software-pipelined at k-chunk-pair
granularity: scores(b2i) -> exp(b2i) on the scalar engine -> weiv(b2i)
trailing one chunk behind, with qk / v-tile units for the NEXT q-tile (or
next batch) interleaved as tensor-engine filler so the PE never idles long
enough for the HAM clock gate to re-throttle. PSUM: scores ring 2x2 banks,
weiv accumulators 2x1, qk/v/rb scratch ring 2x1 = 8 banks.

All weight/bias loads are batched host-side (wp 1 DMA, biases 3, xs 1) and
issued from the sync queue (hardware DGE) instead of gpsimd (software DGE,
~1us per descriptor). gpsimd keeps only memsets, collectives and the a2a
receive DMAs. a2a staging DMAs ride the vector queue right behind the
normalization muls that produce them.

FFN: FFN2 is split 6+2 (not 4+4): 6 column blocks accumulate interleaved
inside the FFN1 loop (PSUM: 2 FFN1 ring + 6 accumulators), the last 2 run
in a short tail that reuses the FFN1 ring slots (no pool-close bubble) and
whose w2 slices are fully preloaded during batch-1 attention.
"""

import sys

for _p in ("/opt/trn_rl_repo",):
    if _p not in sys.path:
        sys.path.append(_p)

import numpy as np
import ml_dtypes

import concourse.bass as bass
import concourse.tile as tile
from concourse import bacc, mybir
from concourse.bass_utils import run_bass_kernel_spmd

BF16 = mybir.dt.bfloat16
F32 = mybir.dt.float32
AF = mybir.ActivationFunctionType
ALU = mybir.AluOpType

N_CORES = 8
B, T, C = 2, 2048, 1024
H, HS = 16, 64
F = 4 * C
TS = 512          # per-core fused shard width (256 tokens x 2 batches)
TSB = 256         # per-batch shard width
CC = C // 128
FB = F // 128
SCALE = float(C) ** -0.5
NA = 6            # FFN2 group-A column blocks (interleaved in FFN1)


def build_program(nc: bass.Bass):
    xt_bf = nc.dram_tensor("xt_bf", [B, CC, 128, T], BF16,
                           kind="ExternalInput").ap()
    xs_d = nc.dram_tensor("xs", [128, CC * TS], F32, kind="ExternalInput").ap()
    wq_d = nc.dram_tensor("wq", [128, C], BF16, kind="ExternalInput").ap()
    wk_d = nc.dram_tensor("wk", [128, C], BF16, kind="ExternalInput").ap()
    wv_d = nc.dram_tensor("wv", [128, C], BF16, kind="ExternalInput").ap()
    wp_d = nc.dram_tensor("wp", [128, 8 * CC * 128], BF16,
                          kind="ExternalInput").ap()
    w1_d = nc.dram_tensor("w1", [CC, 128, F], BF16, kind="ExternalInput").ap()
    w2_d = nc.dram_tensor("w2", [FB, 128, C], BF16, kind="ExternalInput").ap()
    bp_d = nc.dram_tensor("bp", [128, CC], F32, kind="ExternalInput").ap()
    b1_d = nc.dram_tensor("b1", [128, FB], F32, kind="ExternalInput").ap()
    b2_d = nc.dram_tensor("b2", [128, CC], F32, kind="ExternalInput").ap()
    out_d = nc.dram_tensor("outT", [CC, 128, TS], F32, kind="ExternalOutput").ap()

    with tile.TileContext(nc) as tc:
        _emit(nc, tc, xt_bf, xs_d, wq_d, wk_d, wv_d, wp_d, w1_d, w2_d,
              bp_d, b1_d, b2_d, out_d)


def _emit(nc, tc, xt_bf, xs_d, wq_d, wk_d, wv_d, wp_d, w1_d, w2_d,
          bp_d, b1_d, b2_d, out_d):
    from contextlib import ExitStack

    ctx = ExitStack()
    with ctx:
        st = ctx.enter_context(tc.tile_pool(name="static", bufs=1))
        big = ctx.enter_context(tc.tile_pool(name="big", bufs=16))
        expp = ctx.enter_context(tc.tile_pool(name="expp", bufs=4))
        w2p = ctx.enter_context(tc.tile_pool(name="w2s", bufs=3))
        w2bp = ctx.enter_context(tc.tile_pool(name="w2bp", bufs=10))
        outp = ctx.enter_context(tc.tile_pool(name="outp", bufs=2))
        rcp = ctx.enter_context(tc.tile_pool(name="rcp", bufs=1))
        stgp = ctx.enter_context(tc.tile_pool(name="stgp", bufs=2))

        # attention PSUM pools (closed before proj)
        ps_ctx = tc.tile_pool(name="ps", bufs=2, space="PSUM")
        ps = ps_ctx.__enter__()
        sc_ctx = tc.tile_pool(name="scps", bufs=2, space="PSUM")
        scp = sc_ctx.__enter__()
        wv_ctx = tc.tile_pool(name="wvps", bufs=2, space="PSUM")
        wvp = wv_ctx.__enter__()

        a2a_in = [nc.dram_tensor(f"a2a_in{b}", [8 * 128, TSB], BF16,
                                 kind="Internal").ap() for b in range(B)]
        a2a_out = [nc.dram_tensor(f"a2a_out{b}", [8 * 128, TSB], BF16,
                                  kind="Internal").ap() for b in range(B)]
        RG8 = [[0, 1, 2, 3, 4, 5, 6, 7]]

        # ---- gpsimd-cheap setup first: memsets (no DMA deps) ----
        ones1 = st.tile([1, 64], BF16, tag="ones1", name="ones1")
        nc.gpsimd.memset(ones1[:], 1.0)
        mask_big = st.tile([128, 896], BF16, tag="mask", name="mask_big")
        nc.gpsimd.memset(mask_big[:], 1.0)
        nc.gpsimd.affine_select(mask_big[:], mask_big[:], pattern=[[1, 896]],
                                compare_op=ALU.is_ge, fill=0.0, base=-384,
                                channel_multiplier=-1)
        v_sb = [[None] * (T // 128) for _ in range(B)]
        for b in range(B):
            for tk in range(T // 128):
                vt = st.tile([128, 2 * 65], BF16, tag=f"v{b}_{tk}",
                             name=f"v_sb{b}_{tk}")
                nc.gpsimd.memset(vt[:], 1.0)
                v_sb[b][tk] = vt

        # ---- warm-up matmuls: run during the initial input DMA window
        # ---- (mask_big doubles as the warm-up operand) ----
        for wi in range(2):
            acc = ps.tile([128, 512], F32, tag="ps", name=f"wu{wi}")
            for _ in range(18):
                nc.tensor.matmul(acc[:], mask_big[:, 0:128],
                                 mask_big[:, 128:640], start=True, stop=True)

        # ---- input loads: all on the sync queue (hardware DGE) ----
        xt_sb = [[None] * CC for _ in range(B)]
        for b in range(B):
            for cc in range(CC):
                xt_sb[b][cc] = big.tile([128, T], BF16, tag="big",
                                        name=f"xt_sb{b}_{cc}")
        # qkv weights first (small, needed by the very first matmuls),
        # then batch-0 x t-chunk-major so qt=0 work can start early
        wqkv_t = {}
        for nm, d_ in (("k", wk_d), ("q", wq_d), ("v", wv_d)):
            t_ = st.tile([128, C], BF16, tag=f"w{nm}", name=f"w{nm}_t")
            nc.sync.dma_start(t_[:], d_)
            wqkv_t[nm] = t_
        for q4 in range(4):
            for cc in range(CC):
                nc.sync.dma_start(xt_sb[0][cc][:, q4 * 512:(q4 + 1) * 512],
                                  xt_bf[0, cc][:, q4 * 512:(q4 + 1) * 512])
        for cc in range(CC):
            nc.sync.dma_start(xt_sb[1][cc][:], xt_bf[1, cc][:])
        wp_t = st.tile([128, 8 * CC * 128], BF16, tag="wp", name="wp_t")
        nc.sync.dma_start(wp_t[:], wp_d)
        bp_t = st.tile([128, CC], F32, tag="bp", name="bp_t")
        nc.sync.dma_start(bp_t[:], bp_d)
        b1_t = st.tile([128, FB], F32, tag="b1", name="b1_t")
        nc.sync.dma_start(b1_t[:], b1_d)
        b2_t = st.tile([128, CC], F32, tag="b2", name="b2_t")
        nc.sync.dma_start(b2_t[:], b2_d)

        qT = [st.tile([128, T], BF16, tag=f"qT{b}", name=f"qT_sb{b}")
              for b in range(B)]
        kT = [st.tile([128, T], BF16, tag=f"kT{b}", name=f"kT_sb{b}")
              for b in range(B)]
        exq = {}  # (b, qt, hh, b2i) -> exp tile

        # ---------- unit emitters ----------
        def qk_unit(b, nm, tt):
            # one 512-token slice of qT/kT for batch b
            dst = kT[b] if nm == "k" else qT[b]
            acc = ps.tile([128, 512], F32, tag="ps", name=f"pqk_{nm}{b}{tt}")
            w_t = wqkv_t[nm]
            for cc in range(CC):
                nc.tensor.matmul(
                    acc[:],
                    w_t[:, cc * 128:(cc + 1) * 128],
                    xt_sb[b][cc][:, tt * 512:(tt + 1) * 512],
                    start=(cc == 0), stop=(cc == CC - 1),
                )
            nc.vector.tensor_copy(dst[:, tt * 512:(tt + 1) * 512], acc[:])

        def v_unit(b, tk):
            vt = v_sb[b][tk]
            acc = ps.tile([128, 128], F32, tag="ps", name=f"ps_v{b}_{tk}")
            for cc in range(CC):
                nc.tensor.matmul(
                    acc[:],
                    xt_sb[b][cc][:, tk * 128:(tk + 1) * 128],
                    wqkv_t["v"][:, cc * 128:(cc + 1) * 128],
                    start=(cc == 0), stop=(cc == CC - 1),
                )
            src = acc.rearrange("p (h d) -> p h d", h=2)
            dstv = vt.rearrange("p (h d) -> p h d", h=2, d=65)[:, :, 0:64]
            nc.vector.tensor_copy(dstv, src)

        def s_unit(b, qt, b2i):
            # scores + exp (+ causal mask) for k-chunks 2*b2i, 2*b2i+1.
            # The two hh matmuls sit in disjoint PE row groups (partitions
            # 0-63 vs 64-127) so they run concurrently.
            sc = [None, None]
            for hh in range(2):
                sc[hh] = scp.tile([128, 1024], F32, tag="sc",
                                  name=f"psc{b}{hh}{qt}_{b2i}")
            for j in range(2):
                kc = 2 * b2i + j
                for hh in range(2):
                    p0 = 64 * hh
                    nc.tensor.matmul(
                        sc[hh][:, j * 512:(j + 1) * 512],
                        kT[b][p0:p0 + 64, kc * 128:(kc + 1) * 128],
                        qT[b][p0:p0 + 64, qt * 512:(qt + 1) * 512],
                        start=True, stop=True,
                    )
            for hh in range(2):
                ext = expp.tile([128, 1024], BF16, tag="expp",
                                name=f"ex{b}{hh}{qt}_{b2i}")
                nc.scalar.activation(ext[:], sc[hh][:], AF.Exp, scale=SCALE)
                for j in range(2):
                    kc = 2 * b2i + j
                    if kc >= 4 * qt:
                        dd = kc * 128 - qt * 512
                        nc.vector.tensor_mul(
                            ext[:, j * 512:(j + 1) * 512],
                            ext[:, j * 512:(j + 1) * 512],
                            mask_big[:, 384 - dd:896 - dd],
                        )
                exq[(b, qt, hh, b2i)] = ext

        def w_unit(b, qt, b2i, wv_acc):
            nkc = 4 * (qt + 1)
            for j in range(2):
                kc = 2 * b2i + j
                for hh in range(2):
                    ext = exq[(b, qt, hh, b2i)]
                    nc.tensor.matmul(
                        wv_acc[hh][:],
                        v_sb[b][kc][:, hh * 65:hh * 65 + 65],
                        ext[:, j * 512:(j + 1) * 512],
                        start=(kc == 0), stop=(kc == nkc - 1),
                    )

        def f_unit(b, qt, wv_acc):
            # normalize by the softmax denominator (the ones-column of V)
            stg = stgp.tile([128, 512], BF16, tag="stg", name=f"stg{b}{qt}")
            for hh in range(2):
                p0 = 64 * hh
                den = rcp.tile([1, 512], F32, tag="den", name=f"den{b}{hh}{qt}")
                nc.vector.tensor_copy(den[:], wv_acc[hh][64:65, :])
                rc = rcp.tile([1, 512], F32, tag="rc", name=f"rc{b}{hh}{qt}")
                nc.vector.reciprocal_approx_fast(rc[:], den[:])
                rcb = rcp.tile([1, 512], BF16, tag="rcb", name=f"rcb{b}{hh}{qt}")
                nc.vector.tensor_copy(rcb[:], rc[:])
                rb = ps.tile([64, 512], F32, tag="ps", name=f"rb{b}{hh}{qt}")
                nc.tensor.matmul(rb[:], ones1[:], rcb[:], start=True, stop=True)
                rbs = rcp.tile([64, 512], BF16, tag="rbs", name=f"rbs{b}{hh}{qt}")
                nc.vector.tensor_copy(rbs[:], rb[:])
                nc.vector.tensor_mul(stg[p0:p0 + 64, :], wv_acc[hh][0:64, :],
                                     rbs[:])
            # stage the two dest-shard chunks for the AllToAll (gpsimd queue
            # so they don't sit behind weight loads on the sync DMA queue)
            for j in range(2):
                s = 2 * qt + j
                nc.gpsimd.dma_start(
                    a2a_in[b][s * 128:(s + 1) * 128, :],
                    stg[:, j * TSB:(j + 1) * TSB])

        # ---------- attention schedule ----------
        # fillers(b, qt) = prerequisite units of the NEXT qt block, emitted
        # as PE filler between the scalar-gated s/w steps of this block.
        fillers = {
            (0, 0): [("qk", 0, "k", 1), ("qk", 0, "q", 1)] +
                    [("v", 0, tk) for tk in range(4, 8)],
            (0, 1): [("qk", 0, "k", 2), ("qk", 0, "q", 2)] +
                    [("v", 0, tk) for tk in range(8, 12)],
            (0, 2): [("qk", 0, "k", 3), ("qk", 0, "q", 3)] +
                    [("v", 0, tk) for tk in range(12, 16)],
            (0, 3): [("qk", 1, "k", 0), ("qk", 1, "q", 0)] +
                    [("v", 1, tk) for tk in range(0, 4)],
            (1, 0): [("qk", 1, "k", 1), ("qk", 1, "q", 1)] +
                    [("v", 1, tk) for tk in range(4, 8)],
            (1, 1): [("qk", 1, "k", 2), ("qk", 1, "q", 2)] +
                    [("v", 1, tk) for tk in range(8, 12)],
            (1, 2): [("qk", 1, "q", 3), ("v", 1, 12), ("v", 1, 13)],
            (1, 3): [("qk", 1, "k", 3), ("v", 1, 14), ("v", 1, 15)],
        }

        def emit_filler(u):
            if u[0] == "qk":
                qk_unit(u[1], u[2], u[3])
            else:
                v_unit(u[1], u[2])

        def qt_block(b, qt, carry, post=None):
            # software pipeline with a TWO-chunk weiv lag: W(qt,i) is emitted
            # after S(qt,i+2), so the PE always has ~2 exp-latencies of
            # independent work queued ahead of each exp-gated weiv unit. The
            # previous block's last two weiv units (+ its finalize, whose rb
            # broadcast matmuls wait on a short DVE chain) are carried into
            # the head of this block for the same reason.
            nb2i = 2 * (qt + 1)
            fl = list(fillers[(b, qt)])
            # for (1,3) the fillers are this block's OWN late prerequisites:
            # kT(1,3) is only needed from b2i=6, v(1,14/15) from b2i=7.
            own_late = (b, qt) == (1, 3)
            wv_acc = [wvp.tile([65, 512], F32, tag="wv",
                               name=f"pwv{b}{hh}{qt}")
                      for hh in range(2)]
            for i in range(nb2i):
                if own_late:
                    if i == 2 and fl:
                        emit_filler(fl.pop(0))       # kT(1,3)
                    if i == 4 and len(fl) == 2:
                        emit_filler(fl.pop(0))       # v(1,14)
                        emit_filler(fl.pop(0))       # v(1,15)
                s_unit(b, qt, i)
                if carry:
                    carry.pop(0)()
                elif i >= 2:
                    w_unit(b, qt, i - 2, wv_acc)
                if not own_late:
                    if fl:
                        emit_filler(fl.pop(0))
                    if fl and (i <= 1 or len(fl) > nb2i - 1 - i):
                        emit_filler(fl.pop(0))
            while fl:
                emit_filler(fl.pop(0))
            while carry:
                carry.pop(0)()
            if post is not None:
                post()
            return [
                lambda: w_unit(b, qt, nb2i - 2, wv_acc),
                lambda: (w_unit(b, qt, nb2i - 1, wv_acc),
                         f_unit(b, qt, wv_acc)),
            ]

        # batch 0 prerequisites
        qk_unit(0, "k", 0)
        qk_unit(0, "q", 0)
        for tk in range(4):
            v_unit(0, tk)

        carry = []
        for qt in range(4):
            carry = qt_block(0, qt, carry)
        # qt_block(1,0) flushes f(0,3) at its head, so the batch-0 staging
        # DMAs precede the first collective on the gpsimd queue
        carry = qt_block(1, 0, carry)

        nc.gpsimd.collective_compute(
            "AllToAll", ALU.bypass, replica_groups=RG8,
            ins=[a2a_in[0]], outs=[a2a_out[0]],
        )
        rt = [st.tile([128, 8 * TSB], BF16, tag=f"rt{b}", name=f"rt{b}")
              for b in range(B)]

        def emit_rcv(b):
            # one gather-descriptor receive on the sync queue: rt[b][p,
            # s*256+j] <- a2a_out[b][s*128+p, j]. No 8x staggered issue
            # latency, and it never delays the gpsimd staging/collective
            # chain (transfers queued behind it on sync are needed later
            # than the exchange completes).
            nc.sync.dma_start(
                rt[b].rearrange("p (s j) -> p s j", s=8),
                a2a_out[b].rearrange("(s p) j -> p s j", s=8))

        def emit_w1_loads():
            # stream during batch-1 attention on the sync queue
            for half in range(2):
                for cc in range(CC):
                    t_ = big.tile([128, 2048], BF16, tag="big",
                                  name=f"w1_sb{cc}_{half}")
                    nc.sync.dma_start(
                        t_[:], w1_d[cc][:, half * 2048:(half + 1) * 2048])
                    w1_sb[cc][half] = t_

        w1_sb = [[None, None] for _ in range(CC)]

        def post_11():
            emit_rcv(0)
            emit_w1_loads()
            # xs (fp32 residual): right behind w1 on the sync queue, done
            # well before the proj residual-adds need it
            for xi in range(4):
                xst = big.tile([128, 1024], F32, tag="xsb", bufs=4,
                               name=f"xs{xi}")
                nc.sync.dma_start(xst[:], xs_d[:, xi * 1024:(xi + 1) * 1024])
                xs_sb.append(xst)

        xs_sb = []
        carry = qt_block(1, 1, carry, post=post_11)
        carry = qt_block(1, 2, carry)
        carry = qt_block(1, 3, carry)
        while carry:
            carry.pop(0)()  # flush W(1,3,6/7) + f(1,3): the a2a needs them

        wv_ctx.__exit__(None, None, None)
        sc_ctx.__exit__(None, None, None)
        ps_ctx.__exit__(None, None, None)

        # ---- proj (streamed): per cb, 8 matmuls into a ring-2 PSUM tile,
        # residual-add immediately, release. Batch-0 columns only need the
        # first exchange; together with the batch-0 halves of the first NFH
        # FFN1 row-blocks they keep the PE busy for the whole span of the
        # batch-1 AllToAll.
        pf1_ctx = tc.tile_pool(name="pf1", bufs=2, space="PSUM")
        pf1 = pf1_ctx.__enter__()
        pjp_ctx = tc.tile_pool(name="pjp", bufs=2, space="PSUM")
        pjp = pjp_ctx.__enter__()

        def xs_slice(cb):
            return xs_sb[cb // 2][:, (cb % 2) * 512:(cb % 2) * 512 + 512]

        x1b = [st.tile([128, TS], BF16, tag=f"x1b{cb}", name=f"x1b{cb}")
               for cb in range(CC)]

        def proj_cb(cb, h0):
            acc = pjp.tile([128, TSB], F32, tag="pj", name=f"ps_pj{cb}_{h0}")
            for s in range(8):
                nc.tensor.matmul(
                    acc[:],
                    wp_t[:, (s * CC + cb) * 128:(s * CC + cb + 1) * 128],
                    rt[h0][:, s * TSB:(s + 1) * TSB],
                    start=(s == 0), stop=(s == 7),
                )
            nc.vector.scalar_tensor_tensor(
                x1b[cb][:, h0 * TSB:(h0 + 1) * TSB], acc[:],
                bp_t[:, cb:cb + 1],
                xs_slice(cb)[:, h0 * TSB:(h0 + 1) * TSB], ALU.add, ALU.add)

        hT = [st.tile([128, TS], BF16, tag=f"hT{fb}", name=f"hT{fb}")
              for fb in range(FB)]
        # every FFN1 row-block is computed in batch-halves: the batch-0
        # halves (which only need the first exchange) hide the entire
        # batch-1 AllToAll; a half-pair costs the same as one full-width
        # block (the N=256 matmuls still cover the LDWEIGHTS)

        def ffn1_half(fb, h0):
            w1h, fo = fb // 16, fb % 16
            acc = pf1.tile([128, TSB], F32, tag="pf1", name=f"ps_h{fb}_{h0}")
            for cc in range(CC):
                nc.tensor.matmul(
                    acc[:],
                    w1_sb[cc][w1h][:, fo * 128:(fo + 1) * 128],
                    x1b[cc][:, h0 * TSB:(h0 + 1) * TSB],
                    start=(cc == 0), stop=(cc == CC - 1))
            nc.scalar.activation(hT[fb][:, h0 * TSB:(h0 + 1) * TSB], acc[:],
                                 AF.Relu, bias=b1_t[:, fb:fb + 1])

        for cb in range(CC):
            proj_cb(cb, 0)

        nc.gpsimd.collective_compute(
            "AllToAll", ALU.bypass, replica_groups=RG8,
            ins=[a2a_in[1]], outs=[a2a_out[1]],
        )
        emit_rcv(1)

        for fb in range(FB):
            ffn1_half(fb, 0)

        for cb in range(CC):
            proj_cb(cb, 1)
        pjp_ctx.__exit__(None, None, None)

        # ---- FFN1 (remaining) with FFN2 group A (cb 0..NA-1) interleaved
        pfa_ctx = tc.tile_pool(name="pfa", bufs=NA, space="PSUM")
        pfa = pfa_ctx.__enter__()
        accA = [pfa.tile([128, TS], F32, tag=f"pfa{cb}", bufs=1,
                         name=f"ps_oa{cb}") for cb in range(NA)]
        w2b_sb = []

        def ffn2a_mms(fb):
            wt = w2p.tile([128, NA * 128], BF16, tag="w2s", name=f"w2ta{fb}")
            nc.sync.dma_start(wt[:], w2_d[fb][:, 0:NA * 128])
            if fb >= FB - 10:
                # prefetch the first 8 group-B w2 slices during the FFN1 tail
                wtb = w2bp.tile([128, (CC - NA) * 128], BF16, tag="w2b",
                                name=f"w2tb{fb - (FB - 10)}")
                nc.sync.dma_start(wtb[:], w2_d[fb - (FB - 10)][:, NA * 128:C])
                w2b_sb.append(wtb)
            for cb in range(NA):
                nc.tensor.matmul(
                    accA[cb][:], wt[:, cb * 128:(cb + 1) * 128], hT[fb][:],
                    start=(fb == 0), stop=(fb == FB - 1))

        for fb in range(FB):
            ffn1_half(fb, 1)
            ffn2a_mms(fb)

        # ---- FFN2 group B (cb NA..7): reuses the pf1 ring slots; w2
        # slices beyond the prefetched 8 stream just-in-time ----
        accB = [pf1.tile([128, TS], F32, tag="pf1", name=f"ps_ob{cb}")
                for cb in range(CC - NA)]
        first = True
        for fc in range(FB):
            wt = w2b_sb[fc]
            for cb in range(CC - NA):
                nc.tensor.matmul(
                    accB[cb][:], wt[:, cb * 128:(cb + 1) * 128], hT[fc][:],
                    start=(fc == 0), stop=(fc == FB - 1))
            if fc + 10 < FB:
                wtb = w2bp.tile([128, (CC - NA) * 128], BF16, tag="w2b",
                                name=f"w2tb{fc + 10}")
                nc.sync.dma_start(wtb[:], w2_d[fc + 10][:, NA * 128:C])
                w2b_sb.append(wtb)
            if first:
                first = False
                # group-A outputs drain while B accumulates
                for cb in range(NA):
                    ot = outp.tile([128, TS], F32, tag="outp", name=f"ot{cb}")
                    nc.vector.scalar_tensor_tensor(
                        ot[:], accA[cb][:], b2_t[:, cb:cb + 1],
                        x1b[cb][:], ALU.add, ALU.add)
                    nc.scalar.dma_start(out_d[cb], ot[:])
        for cb4 in range(CC - NA):
            cb = cb4 + NA
            ot = outp.tile([128, TS], F32, tag="outp", name=f"ot{cb}")
            nc.vector.scalar_tensor_tensor(ot[:], accB[cb4][:],
                                           b2_t[:, cb:cb + 1],
                                           x1b[cb][:], ALU.add, ALU.add)
            nc.scalar.dma_start(out_d[cb], ot[:])

        pfa_ctx.__exit__(None, None, None)
        pf1_ctx.__exit__(None, None, None)


_CACHED = None


def _get_compiled():
    global _CACHED
    if _CACHED is None:
        nc = bacc.Bacc("TRN2", target_bir_lowering=False, debug=False,
                       num_devices=N_CORES)
        build_program(nc)
        nc.compile()
        _CACHED = nc
    return _CACHED


def _prep_inputs(x, Wq, Wk, Wv, Wproj, bproj, W1, b1, W2, b2):
    bf = ml_dtypes.bfloat16
    W1t = np.ascontiguousarray(W1.astype(bf).reshape(CC, 128, F))
    W2t = np.ascontiguousarray(W2.astype(bf).reshape(FB, 128, C))
    b1r = np.ascontiguousarray(b1.astype(np.float32).reshape(FB, 128).T)
    b2r = np.ascontiguousarray(b2.astype(np.float32).reshape(CC, 128).T)
    bpr = np.ascontiguousarray(bproj.astype(np.float32).reshape(CC, 128).T)
    # full Wproj on every core: wp_flat[p, (s*CC+cb)*128+k] = Wproj[128s+p, 128cb+k]
    wp_flat = np.ascontiguousarray(
        Wproj.astype(bf).reshape(8, 128, CC, 128).transpose(1, 0, 2, 3)
        .reshape(128, 8 * CC * 128))
    # x transposed, both batches, shared by all cores
    xT = [np.ascontiguousarray(x[b].T.astype(np.float32)) for b in range(B)]
    xT_bf = np.ascontiguousarray(
        np.stack([xT[b].astype(bf).reshape(CC, 128, T) for b in range(B)]))

    in_maps = []
    for c in range(N_CORES):
        cols = slice(128 * c, 128 * (c + 1))
        # wq_t[p, cc*128+k] = Wq[128cc+p, core_cols[k]]
        wq_s = np.ascontiguousarray(
            Wq[:, cols].astype(bf).reshape(CC, 128, 128)
            .transpose(1, 0, 2).reshape(128, C))
        wk_s = np.ascontiguousarray(
            Wk[:, cols].astype(bf).reshape(CC, 128, 128)
            .transpose(1, 0, 2).reshape(128, C))
        wv_s = np.ascontiguousarray(
            Wv[:, cols].astype(bf).reshape(CC, 128, 128)
            .transpose(1, 0, 2).reshape(128, C))
        tok = slice(TSB * c, TSB * (c + 1))
        # xs_t[p, cb*512+j] = fused-shard residual, fp32
        xts = np.ascontiguousarray(
            np.concatenate([xT[0][:, tok], xT[1][:, tok]], axis=1)
            .reshape(CC, 128, TS).transpose(1, 0, 2).reshape(128, CC * TS))
        in_maps.append({
            "xt_bf": xT_bf, "xs": xts,
            "wq": wq_s, "wk": wk_s, "wv": wv_s, "wp": wp_flat,
            "w1": W1t, "w2": W2t, "bp": bpr, "b1": b1r, "b2": b2r,
        })
    return in_maps


def kernel(x, Wq, Wk, Wv, Wproj, bproj, W1, b1, W2, b2, _trace=False):
    nc = _get_compiled()
    in_maps = _prep_inputs(np.asarray(x), np.asarray(Wq), np.asarray(Wk),
                           np.asarray(Wv), np.asarray(Wproj), np.asarray(bproj),
                           np.asarray(W1), np.asarray(b1), np.asarray(W2),
                           np.asarray(b2))
    res = run_bass_kernel_spmd(nc, in_maps, list(range(N_CORES)), trace=_trace)
    out = np.empty((B, T, C), dtype=np.float32)
    for c in range(N_CORES):
        shard = res.results[c]["outT"].reshape(C, TS)
        for b in range(B):
            out[b, TSB * c: TSB * (c + 1), :] = shard[:, TSB * b:TSB * (b + 1)].T
    if _trace:
        kernel.last_exec_time_ns = res.exec_time_ns
    return out


# revision 32
# speedup vs baseline: 1.0448x; 1.0242x over previous
"""Fused transformer block (attention + MLP) on 8 trn2 NeuronCores.

Sharding (8-way, batch-symmetric): every core computes attention for ONE
head-pair (heads 2c, 2c+1) of BOTH batches, and owns a 256-token shard of
each batch (tokens [256c, 256c+256)). The two shard halves are concatenated
along the free axis into one 512-column tile set, so projection + FFN code
is identical to a single 512-token shard.

Schedule: the attention inner loop is software-pipelined at k-chunk-pair
granularity: scores(b2i) -> exp(b2i) on the scalar engine -> weiv(b2i)
trailing one chunk behind, with qk / v-tile units for the NEXT q-tile (or
next batch) interleaved as tensor-engine filler so the PE never idles long
enough for the HAM clock gate to re-throttle. PSUM: scores ring 2x2 banks,
weiv accumulators 2x1, qk/v/rb scratch ring 2x1 = 8 banks.

All weight/bias loads are batched host-side (wp 1 DMA, biases 3, xs 1) and
issued from the sync queue (hardware DGE) instead of gpsimd (software DGE,
~1us per descriptor). gpsimd keeps only memsets, the collectives and the
a2a staging DMAs; the exchange receives are single gather-descriptor DMAs
on the sync queue into one wide tile per batch.

Proj is streamed (ring-2 PSUM, residual-add per cb, release), and every
FFN1 row-block is computed as two N=256 batch-halves: the batch-0 halves
need only the first exchange, so they hide the entire batch-1 AllToAll,
and N=256 matmuls still cover their LDWEIGHTS (a half-pair costs the same
as one full-width block). FFN2 is split 6+2: 6 column blocks accumulate
interleaved inside the FFN1-half1 loop (PSUM: 2 FFN1 ring + 6
accumulators), the last 2 run in a short tail that reuses the FFN1 ring
slots, with their w2 slices prefetched/streamed just-in-time.
"""

import sys

for _p in ("/opt/trn_rl_repo",):
    if _p not in sys.path:
        sys.path.append(_p)

import numpy as np
import ml_dtypes

import concourse.bass as bass
import concourse.tile as tile
from concourse import bacc, mybir
from concourse.bass_utils import run_bass_kernel_spmd

BF16 = mybir.dt.bfloat16
F32 = mybir.dt.float32
AF = mybir.ActivationFunctionType
ALU = mybir.AluOpType

N_CORES = 8
B, T, C = 2, 2048, 1024
H, HS = 16, 64
F = 4 * C
TS = 512          # per-core fused shard width (256 tokens x 2 batches)
TSB = 256         # per-batch shard width
CC = C // 128
FB = F // 128
SCALE = float(C) ** -0.5
NA = 6            # FFN2 group-A column blocks (interleaved in FFN1)


def build_program(nc: bass.Bass):
    xt_bf = nc.dram_tensor("xt_bf", [B, CC, 128, T], BF16,
                           kind="ExternalInput").ap()
    xs_d = nc.dram_tensor("xs", [128, CC * TS], F32, kind="ExternalInput").ap()
    wq_d = nc.dram_tensor("wq", [128, C], BF16, kind="ExternalInput").ap()
    wk_d = nc.dram_tensor("wk", [128, C], BF16, kind="ExternalInput").ap()
    wv_d = nc.dram_tensor("wv", [128, C], BF16, kind="ExternalInput").ap()
    wp_d = nc.dram_tensor("wp", [128, 8 * CC * 128], BF16,
                          kind="ExternalInput").ap()
    w1_d = nc.dram_tensor("w1", [CC, 128, F], BF16, kind="ExternalInput").ap()
    w2_d = nc.dram_tensor("w2", [FB, 128, C], BF16, kind="ExternalInput").ap()
    bp_d = nc.dram_tensor("bp", [128, CC], F32, kind="ExternalInput").ap()
    b1_d = nc.dram_tensor("b1", [128, FB], F32, kind="ExternalInput").ap()
    b2_d = nc.dram_tensor("b2", [128, CC], F32, kind="ExternalInput").ap()
    out_d = nc.dram_tensor("outT", [CC, 128, TS], F32, kind="ExternalOutput").ap()

    with tile.TileContext(nc) as tc:
        _emit(nc, tc, xt_bf, xs_d, wq_d, wk_d, wv_d, wp_d, w1_d, w2_d,
              bp_d, b1_d, b2_d, out_d)


def _emit(nc, tc, xt_bf, xs_d, wq_d, wk_d, wv_d, wp_d, w1_d, w2_d,
          bp_d, b1_d, b2_d, out_d):
    from contextlib import ExitStack

    ctx = ExitStack()
    with ctx:
        st = ctx.enter_context(tc.tile_pool(name="static", bufs=1))
        big = ctx.enter_context(tc.tile_pool(name="big", bufs=16))
        expp = ctx.enter_context(tc.tile_pool(name="expp", bufs=2))
        w2p = ctx.enter_context(tc.tile_pool(name="w2s", bufs=3))
        w2bp = ctx.enter_context(tc.tile_pool(name="w2bp", bufs=10))
        outp = ctx.enter_context(tc.tile_pool(name="outp", bufs=2))
        rcp = ctx.enter_context(tc.tile_pool(name="rcp", bufs=1))
        stgp = ctx.enter_context(tc.tile_pool(name="stgp", bufs=2))

        # attention PSUM pools (closed before proj)
        ps_ctx = tc.tile_pool(name="ps", bufs=2, space="PSUM")
        ps = ps_ctx.__enter__()
        sc_ctx = tc.tile_pool(name="scps", bufs=1, space="PSUM")
        scp = sc_ctx.__enter__()
        wv_ctx = tc.tile_pool(name="wvps", bufs=2, space="PSUM")
        wvp = wv_ctx.__enter__()

        a2a_in = [nc.dram_tensor(f"a2a_in{b}", [8 * 128, TSB], BF16,
                                 kind="Internal").ap() for b in range(B)]
        a2a_out = [nc.dram_tensor(f"a2a_out{b}", [8 * 128, TSB], BF16,
                                  kind="Internal").ap() for b in range(B)]
        RG8 = [[0, 1, 2, 3, 4, 5, 6, 7]]

        # ---- gpsimd-cheap setup first: memsets (no DMA deps) ----
        ones1 = st.tile([1, 64], BF16, tag="ones1", name="ones1")
        nc.gpsimd.memset(ones1[:], 1.0)
        mask_big = st.tile([128, 896], BF16, tag="mask", name="mask_big")
        nc.gpsimd.memset(mask_big[:], 1.0)
        nc.gpsimd.affine_select(mask_big[:], mask_big[:], pattern=[[1, 896]],
                                compare_op=ALU.is_ge, fill=0.0, base=-384,
                                channel_multiplier=-1)
        v_sb = [[None] * (T // 128) for _ in range(B)]
        for b in range(B):
            for tk in range(T // 128):
                vt = st.tile([128, 2 * 65], BF16, tag=f"v{b}_{tk}",
                             name=f"v_sb{b}_{tk}")
                nc.gpsimd.memset(vt[:], 1.0)
                v_sb[b][tk] = vt

        # ---- warm-up matmuls: run during the initial input DMA window
        # ---- (mask_big doubles as the warm-up operand) ----
        for wi in range(2):
            acc = ps.tile([128, 512], F32, tag="ps", name=f"wu{wi}")
            for _ in range(18):
                nc.tensor.matmul(acc[:], mask_big[:, 0:128],
                                 mask_big[:, 128:640], start=True, stop=True)

        # ---- input loads: all on the sync queue (hardware DGE) ----
        xt_sb = [[None] * CC for _ in range(B)]
        for b in range(B):
            for cc in range(CC):
                xt_sb[b][cc] = big.tile([128, T], BF16, tag="big",
                                        name=f"xt_sb{b}_{cc}")
        # qkv weights first (small, needed by the very first matmuls),
        # then batch-0 x t-chunk-major so qt=0 work can start early
        wqkv_t = {}
        for nm, d_ in (("k", wk_d), ("q", wq_d), ("v", wv_d)):
            t_ = st.tile([128, C], BF16, tag=f"w{nm}", name=f"w{nm}_t")
            nc.sync.dma_start(t_[:], d_)
            wqkv_t[nm] = t_
        for q4 in range(4):
            for cc in range(CC):
                nc.sync.dma_start(xt_sb[0][cc][:, q4 * 512:(q4 + 1) * 512],
                                  xt_bf[0, cc][:, q4 * 512:(q4 + 1) * 512])
        for cc in range(CC):
            nc.sync.dma_start(xt_sb[1][cc][:], xt_bf[1, cc][:])
        wp_t = st.tile([128, 8 * CC * 128], BF16, tag="wp", name="wp_t")
        nc.sync.dma_start(wp_t[:], wp_d)
        bp_t = st.tile([128, CC], F32, tag="bp", name="bp_t")
        nc.sync.dma_start(bp_t[:], bp_d)
        b1_t = st.tile([128, FB], F32, tag="b1", name="b1_t")
        nc.sync.dma_start(b1_t[:], b1_d)
        b2_t = st.tile([128, CC], F32, tag="b2", name="b2_t")
        nc.sync.dma_start(b2_t[:], b2_d)

        qT = [st.tile([128, T], BF16, tag=f"qT{b}", name=f"qT_sb{b}")
              for b in range(B)]
        kT = [st.tile([128, T], BF16, tag=f"kT{b}", name=f"kT_sb{b}")
              for b in range(B)]
        exq = {}  # (b, qt, b2i) -> fused exp tile (both heads)

        # ---------- unit emitters ----------
        def qk_unit(b, nm, tt):
            # one 512-token slice of qT/kT for batch b
            dst = kT[b] if nm == "k" else qT[b]
            acc = ps.tile([128, 512], F32, tag="ps", name=f"pqk_{nm}{b}{tt}")
            w_t = wqkv_t[nm]
            for cc in range(CC):
                nc.tensor.matmul(
                    acc[:],
                    w_t[:, cc * 128:(cc + 1) * 128],
                    xt_sb[b][cc][:, tt * 512:(tt + 1) * 512],
                    start=(cc == 0), stop=(cc == CC - 1),
                )
            nc.vector.tensor_copy(dst[:, tt * 512:(tt + 1) * 512], acc[:])

        def v_unit(b, tk):
            vt = v_sb[b][tk]
            acc = ps.tile([128, 128], F32, tag="ps", name=f"ps_v{b}_{tk}")
            for cc in range(CC):
                nc.tensor.matmul(
                    acc[:],
                    xt_sb[b][cc][:, tk * 128:(tk + 1) * 128],
                    wqkv_t["v"][:, cc * 128:(cc + 1) * 128],
                    start=(cc == 0), stop=(cc == CC - 1),
                )
            src = acc.rearrange("p (h d) -> p h d", h=2)
            dstv = vt.rearrange("p (h d) -> p h d", h=2, d=65)[:, :, 0:64]
            nc.vector.tensor_copy(dstv, src)

        def s_unit(b, qt, b2i):
            # scores + exp (+ causal mask) for k-chunks 2*b2i, 2*b2i+1.
            # Both heads land in ONE 4-bank psum tile so a single fused
            # [128,2048] exp covers them (half the scalar-queue overhead).
            # The two hh matmuls sit in disjoint PE row groups (partitions
            # 0-63 vs 64-127) so they run concurrently.
            sc = scp.tile([128, 2048], F32, tag="sc", name=f"psc{b}{qt}_{b2i}")
            for j in range(2):
                kc = 2 * b2i + j
                for hh in range(2):
                    p0 = 64 * hh
                    nc.tensor.matmul(
                        sc[:, (hh * 2 + j) * 512:(hh * 2 + j + 1) * 512],
                        kT[b][p0:p0 + 64, kc * 128:(kc + 1) * 128],
                        qT[b][p0:p0 + 64, qt * 512:(qt + 1) * 512],
                        start=True, stop=True,
                    )
            ext = expp.tile([128, 2048], BF16, tag="expp",
                            name=f"ex{b}{qt}_{b2i}")
            nc.scalar.activation(ext[:], sc[:], AF.Exp, scale=SCALE)
            for hh in range(2):
                for j in range(2):
                    kc = 2 * b2i + j
                    if kc >= 4 * qt:
                        dd = kc * 128 - qt * 512
                        nc.vector.tensor_mul(
                            ext[:, (hh * 2 + j) * 512:(hh * 2 + j + 1) * 512],
                            ext[:, (hh * 2 + j) * 512:(hh * 2 + j + 1) * 512],
                            mask_big[:, 384 - dd:896 - dd],
                        )
            exq[(b, qt, b2i)] = ext

        def w_unit(b, qt, b2i, wv_acc):
            nkc = 4 * (qt + 1)
            ext = exq[(b, qt, b2i)]
            for j in range(2):
                kc = 2 * b2i + j
                for hh in range(2):
                    nc.tensor.matmul(
                        wv_acc[hh][:],
                        v_sb[b][kc][:, hh * 65:hh * 65 + 65],
                        ext[:, (hh * 2 + j) * 512:(hh * 2 + j + 1) * 512],
                        start=(kc == 0), stop=(kc == nkc - 1),
                    )

        def f_unit(b, qt, wv_acc):
            # normalize by the softmax denominator (the ones-column of V)
            stg = stgp.tile([128, 512], BF16, tag="stg", name=f"stg{b}{qt}")
            for hh in range(2):
                p0 = 64 * hh
                den = rcp.tile([1, 512], F32, tag="den", name=f"den{b}{hh}{qt}")
                nc.vector.tensor_copy(den[:], wv_acc[hh][64:65, :])
                rc = rcp.tile([1, 512], F32, tag="rc", name=f"rc{b}{hh}{qt}")
                nc.vector.reciprocal_approx_fast(rc[:], den[:])
                rcb = rcp.tile([1, 512], BF16, tag="rcb", name=f"rcb{b}{hh}{qt}")
                nc.vector.tensor_copy(rcb[:], rc[:])
                rb = ps.tile([64, 512], F32, tag="ps", name=f"rb{b}{hh}{qt}")
                nc.tensor.matmul(rb[:], ones1[:], rcb[:], start=True, stop=True)
                rbs = rcp.tile([64, 512], BF16, tag="rbs", name=f"rbs{b}{hh}{qt}")
                nc.vector.tensor_copy(rbs[:], rb[:])
                nc.vector.tensor_mul(stg[p0:p0 + 64, :], wv_acc[hh][0:64, :],
                                     rbs[:])
            # stage the two dest-shard chunks for the AllToAll (gpsimd queue
            # so they don't sit behind weight loads on the sync DMA queue)
            for j in range(2):
                s = 2 * qt + j
                nc.gpsimd.dma_start(
                    a2a_in[b][s * 128:(s + 1) * 128, :],
                    stg[:, j * TSB:(j + 1) * TSB])

        # ---------- attention schedule ----------
        # fillers(b, qt) = prerequisite units of the NEXT qt block, emitted
        # as PE filler between the scalar-gated s/w steps of this block.
        fillers = {
            (0, 0): [("qk", 0, "k", 1), ("qk", 0, "q", 1)] +
                    [("v", 0, tk) for tk in range(4, 8)],
            (0, 1): [("qk", 0, "k", 2), ("qk", 0, "q", 2)] +
                    [("v", 0, tk) for tk in range(8, 12)],
            (0, 2): [("qk", 0, "k", 3), ("qk", 0, "q", 3)] +
                    [("v", 0, tk) for tk in range(12, 16)],
            (0, 3): [("qk", 1, "k", 0), ("qk", 1, "q", 0)] +
                    [("v", 1, tk) for tk in range(0, 4)],
            (1, 0): [("qk", 1, "k", 1), ("qk", 1, "q", 1)] +
                    [("v", 1, tk) for tk in range(4, 8)],
            (1, 1): [("qk", 1, "k", 2), ("qk", 1, "q", 2)] +
                    [("v", 1, tk) for tk in range(8, 12)],
            (1, 2): [("qk", 1, "q", 3), ("v", 1, 12), ("v", 1, 13)],
            (1, 3): [("qk", 1, "k", 3), ("v", 1, 14), ("v", 1, 15)],
        }

        def emit_filler(u):
            if u[0] == "qk":
                qk_unit(u[1], u[2], u[3])
            else:
                v_unit(u[1], u[2])

        def qt_block(b, qt, carry, post=None):
            # software pipeline with a TWO-chunk weiv lag: W(qt,i) is emitted
            # after S(qt,i+2), so the PE always has ~2 exp-latencies of
            # independent work queued ahead of each exp-gated weiv unit. The
            # previous block's last two weiv units (+ its finalize, whose rb
            # broadcast matmuls wait on a short DVE chain) are carried into
            # the head of this block for the same reason.
            nb2i = 2 * (qt + 1)
            fl = list(fillers[(b, qt)])
            # for (1,3) the fillers are this block's OWN late prerequisites:
            # kT(1,3) is only needed from b2i=6, v(1,14/15) from b2i=7.
            own_late = (b, qt) == (1, 3)
            wv_acc = [wvp.tile([65, 512], F32, tag="wv",
                               name=f"pwv{b}{hh}{qt}")
                      for hh in range(2)]
            for i in range(nb2i):
                if own_late:
                    if i == 2 and fl:
                        emit_filler(fl.pop(0))       # kT(1,3)
                    if i == 4 and len(fl) == 2:
                        emit_filler(fl.pop(0))       # v(1,14)
                        emit_filler(fl.pop(0))       # v(1,15)
                s_unit(b, qt, i)
                if carry:
                    carry.pop(0)()
                elif i >= 2:
                    w_unit(b, qt, i - 2, wv_acc)
                if not own_late:
                    if fl:
                        emit_filler(fl.pop(0))
                    if fl and (i <= 1 or len(fl) > nb2i - 1 - i):
                        emit_filler(fl.pop(0))
            while fl:
                emit_filler(fl.pop(0))
            while carry:
                carry.pop(0)()
            if post is not None:
                post()
            return [
                lambda: w_unit(b, qt, nb2i - 2, wv_acc),
                lambda: (w_unit(b, qt, nb2i - 1, wv_acc),
                         f_unit(b, qt, wv_acc)),
            ]

        # batch 0 prerequisites
        qk_unit(0, "k", 0)
        qk_unit(0, "q", 0)
        for tk in range(4):
            v_unit(0, tk)

        carry = []
        for qt in range(4):
            carry = qt_block(0, qt, carry)
        # qt_block(1,0) flushes f(0,3) at its head, so the batch-0 staging
        # DMAs precede the first collective on the gpsimd queue
        carry = qt_block(1, 0, carry)

        nc.gpsimd.collective_compute(
            "AllToAll", ALU.bypass, replica_groups=RG8,
            ins=[a2a_in[0]], outs=[a2a_out[0]],
        )
        rt = [st.tile([128, 8 * TSB], BF16, tag=f"rt{b}", name=f"rt{b}")
              for b in range(B)]

        def emit_rcv(b):
            # one gather-descriptor receive on the sync queue: rt[b][p,
            # s*256+j] <- a2a_out[b][s*128+p, j]. No 8x staggered issue
            # latency, and it never delays the gpsimd staging/collective
            # chain (transfers queued behind it on sync are needed later
            # than the exchange completes).
            nc.sync.dma_start(
                rt[b].rearrange("p (s j) -> p s j", s=8),
                a2a_out[b].rearrange("(s p) j -> p s j", s=8))

        def emit_w1_loads():
            # stream during batch-1 attention on the sync queue
            for half in range(2):
                for cc in range(CC):
                    t_ = big.tile([128, 2048], BF16, tag="big",
                                  name=f"w1_sb{cc}_{half}")
                    nc.sync.dma_start(
                        t_[:], w1_d[cc][:, half * 2048:(half + 1) * 2048])
                    w1_sb[cc][half] = t_

        w1_sb = [[None, None] for _ in range(CC)]

        def post_11():
            emit_rcv(0)
            emit_w1_loads()
            # xs (fp32 residual): right behind w1 on the sync queue, done
            # well before the proj residual-adds need it
            for xi in range(4):
                xst = big.tile([128, 1024], F32, tag="xsb", bufs=4,
                               name=f"xs{xi}")
                nc.sync.dma_start(xst[:], xs_d[:, xi * 1024:(xi + 1) * 1024])
                xs_sb.append(xst)

        xs_sb = []
        carry = qt_block(1, 1, carry, post=post_11)
        carry = qt_block(1, 2, carry)
        carry = qt_block(1, 3, carry)
        while carry:
            carry.pop(0)()  # flush W(1,3,6/7) + f(1,3): the a2a needs them

        wv_ctx.__exit__(None, None, None)
        sc_ctx.__exit__(None, None, None)
        ps_ctx.__exit__(None, None, None)

        # ---- proj (streamed): per cb, 8 matmuls into a ring-2 PSUM tile,
        # residual-add immediately, release. Batch-0 columns only need the
        # first exchange; together with the batch-0 halves of the FFN1
        # row-blocks they keep the PE busy for the whole span of the
        # batch-1 AllToAll.
        pf1_ctx = tc.tile_pool(name="pf1", bufs=2, space="PSUM")
        pf1 = pf1_ctx.__enter__()
        pjp_ctx = tc.tile_pool(name="pjp", bufs=2, space="PSUM")
        pjp = pjp_ctx.__enter__()

        def xs_slice(cb):
            return xs_sb[cb // 2][:, (cb % 2) * 512:(cb % 2) * 512 + 512]

        x1b = [st.tile([128, TS], BF16, tag=f"x1b{cb}", name=f"x1b{cb}")
               for cb in range(CC)]

        def proj_cb(cb, h0):
            acc = pjp.tile([128, TSB], F32, tag="pj", name=f"ps_pj{cb}_{h0}")
            for s in range(8):
                nc.tensor.matmul(
                    acc[:],
                    wp_t[:, (s * CC + cb) * 128:(s * CC + cb + 1) * 128],
                    rt[h0][:, s * TSB:(s + 1) * TSB],
                    start=(s == 0), stop=(s == 7),
                )
            nc.vector.scalar_tensor_tensor(
                x1b[cb][:, h0 * TSB:(h0 + 1) * TSB], acc[:],
                bp_t[:, cb:cb + 1],
                xs_slice(cb)[:, h0 * TSB:(h0 + 1) * TSB], ALU.add, ALU.add)

        hT = [st.tile([128, TS], BF16, tag=f"hT{fb}", name=f"hT{fb}")
              for fb in range(FB)]
        # every FFN1 row-block is computed in batch-halves: the batch-0
        # halves (which only need the first exchange) hide the entire
        # batch-1 AllToAll; a half-pair costs the same as one full-width
        # block (the N=256 matmuls still cover the LDWEIGHTS)

        def ffn1_half(fb, h0):
            w1h, fo = fb // 16, fb % 16
            acc = pf1.tile([128, TSB], F32, tag="pf1", name=f"ps_h{fb}_{h0}")
            for cc in range(CC):
                nc.tensor.matmul(
                    acc[:],
                    w1_sb[cc][w1h][:, fo * 128:(fo + 1) * 128],
                    x1b[cc][:, h0 * TSB:(h0 + 1) * TSB],
                    start=(cc == 0), stop=(cc == CC - 1))
            nc.scalar.activation(hT[fb][:, h0 * TSB:(h0 + 1) * TSB], acc[:],
                                 AF.Relu, bias=b1_t[:, fb:fb + 1])

        for cb in range(CC):
            proj_cb(cb, 0)

        nc.gpsimd.collective_compute(
            "AllToAll", ALU.bypass, replica_groups=RG8,
            ins=[a2a_in[1]], outs=[a2a_out[1]],
        )
        emit_rcv(1)

        for fb in range(FB):
            ffn1_half(fb, 0)

        for cb in range(CC):
            proj_cb(cb, 1)
        pjp_ctx.__exit__(None, None, None)

        # ---- FFN1 (remaining) with FFN2 group A (cb 0..NA-1) interleaved
        pfa_ctx = tc.tile_pool(name="pfa", bufs=NA, space="PSUM")
        pfa = pfa_ctx.__enter__()
        accA = [pfa.tile([128, TS], F32, tag=f"pfa{cb}", bufs=1,
                         name=f"ps_oa{cb}") for cb in range(NA)]
        w2b_sb = []

        def ffn2a_mms(fb):
            wt = w2p.tile([128, NA * 128], BF16, tag="w2s", name=f"w2ta{fb}")
            nc.sync.dma_start(wt[:], w2_d[fb][:, 0:NA * 128])
            if fb >= FB - 10:
                # prefetch the first 8 group-B w2 slices during the FFN1 tail
                wtb = w2bp.tile([128, (CC - NA) * 128], BF16, tag="w2b",
                                name=f"w2tb{fb - (FB - 10)}")
                nc.sync.dma_start(wtb[:], w2_d[fb - (FB - 10)][:, NA * 128:C])
                w2b_sb.append(wtb)
            for cb in range(NA):
                nc.tensor.matmul(
                    accA[cb][:], wt[:, cb * 128:(cb + 1) * 128], hT[fb][:],
                    start=(fb == 0), stop=(fb == FB - 1))

        for fb in range(FB):
            ffn1_half(fb, 1)
            ffn2a_mms(fb)

        # ---- FFN2 group B (cb NA..7): reuses the pf1 ring slots; w2
        # slices beyond the prefetched 8 stream just-in-time ----
        accB = [pf1.tile([128, TS], F32, tag="pf1", name=f"ps_ob{cb}")
                for cb in range(CC - NA)]
        first = True
        for fc in range(FB):
            wt = w2b_sb[fc]
            for cb in range(CC - NA):
                nc.tensor.matmul(
                    accB[cb][:], wt[:, cb * 128:(cb + 1) * 128], hT[fc][:],
                    start=(fc == 0), stop=(fc == FB - 1))
            if fc + 10 < FB:
                wtb = w2bp.tile([128, (CC - NA) * 128], BF16, tag="w2b",
                                name=f"w2tb{fc + 10}")
                nc.sync.dma_start(wtb[:], w2_d[fc + 10][:, NA * 128:C])
                w2b_sb.append(wtb)
            if first:
                first = False
                # group-A outputs drain while B accumulates
                for cb in range(NA):
                    ot = outp.tile([128, TS], F32, tag="outp", name=f"ot{cb}")
                    nc.vector.scalar_tensor_tensor(
                        ot[:], accA[cb][:], b2_t[:, cb:cb + 1],
                        x1b[cb][:], ALU.add, ALU.add)
                    nc.scalar.dma_start(out_d[cb], ot[:])
        for cb4 in range(CC - NA):
            cb = cb4 + NA
            ot = outp.tile([128, TS], F32, tag="outp", name=f"ot{cb}")
            nc.vector.scalar_tensor_tensor(ot[:], accB[cb4][:],
                                           b2_t[:, cb:cb + 1],
                                           x1b[cb][:], ALU.add, ALU.add)
            nc.scalar.dma_start(out_d[cb], ot[:])

        pfa_ctx.__exit__(None, None, None)
        pf1_ctx.__exit__(None, None, None)


_CACHED = None


def _get_compiled():
    global _CACHED
    if _CACHED is None:
        nc = bacc.Bacc("TRN2", target_bir_lowering=False, debug=False,
                       num_devices=N_CORES)
        build_program(nc)
        nc.compile()
        _CACHED = nc
    return _CACHED


def _prep_inputs(x, Wq, Wk, Wv, Wproj, bproj, W1, b1, W2, b2):
    bf = ml_dtypes.bfloat16
    W1t = np.ascontiguousarray(W1.astype(bf).reshape(CC, 128, F))
    W2t = np.ascontiguousarray(W2.astype(bf).reshape(FB, 128, C))
    b1r = np.ascontiguousarray(b1.astype(np.float32).reshape(FB, 128).T)
    b2r = np.ascontiguousarray(b2.astype(np.float32).reshape(CC, 128).T)
    bpr = np.ascontiguousarray(bproj.astype(np.float32).reshape(CC, 128).T)
    # full Wproj on every core: wp_flat[p, (s*CC+cb)*128+k] = Wproj[128s+p, 128cb+k]
    wp_flat = np.ascontiguousarray(
        Wproj.astype(bf).reshape(8, 128, CC, 128).transpose(1, 0, 2, 3)
        .reshape(128, 8 * CC * 128))
    # x transposed, both batches, shared by all cores
    xT = [np.ascontiguousarray(x[b].T.astype(np.float32)) for b in range(B)]
    xT_bf = np.ascontiguousarray(
        np.stack([xT[b].astype(bf).reshape(CC, 128, T) for b in range(B)]))

    in_maps = []
    for c in range(N_CORES):
        cols = slice(128 * c, 128 * (c + 1))
        # wq_t[p, cc*128+k] = Wq[128cc+p, core_cols[k]]
        wq_s = np.ascontiguousarray(
            Wq[:, cols].astype(bf).reshape(CC, 128, 128)
            .transpose(1, 0, 2).reshape(128, C))
        wk_s = np.ascontiguousarray(
            Wk[:, cols].astype(bf).reshape(CC, 128, 128)
            .transpose(1, 0, 2).reshape(128, C))
        wv_s = np.ascontiguousarray(
            Wv[:, cols].astype(bf).reshape(CC, 128, 128)
            .transpose(1, 0, 2).reshape(128, C))
        tok = slice(TSB * c, TSB * (c + 1))
        # xs_t[p, cb*512+j] = fused-shard residual, fp32
        xts = np.ascontiguousarray(
            np.concatenate([xT[0][:, tok], xT[1][:, tok]], axis=1)
            .reshape(CC, 128, TS).transpose(1, 0, 2).reshape(128, CC * TS))
        in_maps.append({
            "xt_bf": xT_bf, "xs": xts,
            "wq": wq_s, "wk": wk_s, "wv": wv_s, "wp": wp_flat,
            "w1": W1t, "w2": W2t, "bp": bpr, "b1": b1r, "b2": b2r,
        })
    return in_maps


def kernel(x, Wq, Wk, Wv, Wproj, bproj, W1, b1, W2, b2, _trace=False):
    nc = _get_compiled()
    in_maps = _prep_inputs(np.asarray(x), np.asarray(Wq), np.asarray(Wk),
                           np.asarray(Wv), np.asarray(Wproj), np.asarray(bproj),
                           np.asarray(W1), np.asarray(b1), np.asarray(W2),
                           np.asarray(b2))
    res = run_bass_kernel_spmd(nc, in_maps, list(range(N_CORES)), trace=_trace)
    out = np.empty((B, T, C), dtype=np.float32)
    for c in range(N_CORES):
        shard = res.results[c]["outT"].reshape(C, TS)
        for b in range(B):
            out[b, TSB * c: TSB * (c + 1), :] = shard[:, TSB * b:TSB * (b + 1)].T
    if _trace:
        kernel.last_exec_time_ns = res.exec_time_ns
    return out
